# revision 4
# baseline (speedup 1.0000x reference)
"""JacobiGNN v11: AllGather split into two 8KB halves; GEMM2 consumes
the first half while the second is in flight.
Was v9: Jacobi on vector (gpsimd queue = CC path only),
z copies all on scalar.
Was v8: batched per-half softmax tail, Ysb on sync queue,
split ag_in bounce.
Was v7: z-GEMM direct to [node,C] (no z transposes; b2 folded
as host-side rank-1 correction into the G descale), split softmax tail.
Was v6: v5 but all bulk DMAs back on the sync queue.
Was v5: MLP DoubleRow, warm AG back, DMA descgen spread.
Was v4: DoubleRow e4m3 GEMMs, x on sync queue first, split tail.
Was v3: v2 + packed DMA layouts (8KB lines), no warm AG,
wide ag bounce layout, batched z copies.

Per core i (SPMD, rank-agnostic program; shards differ via inputs):
  u1 = U[:, cols_i]*Su       [N, SH] fp8  -> GEMM1 rhs (G^T = z^T @ U_cols)
  u2 = U[rows_i, :].T*Su     [N, SH] fp8  -> GEMM2 rhs (out^T = Y^T @ U_rows^T)
  MLP replicated (x^T fp8), z*Sz fp8 weights; Jacobi filter on gpsimd.
  y_i = (Hfilt*Sy) * G_i -> fp8; AllGather(y) -> Y full; GEMM2 local.
  log_softmax over C on-chip; out rows_i exact.
Scales: Su=64, Sz=8, Sy=4, Sx=2, Sw=64; descale folded into identities
(idg = I/(Su*Sz), ido = I/(Su*Sy)) and the MLP activation scale.
"""

import os
import sys

import numpy as np

for _p in ("/opt/trn_rl_repo", "/root/.axon_site/_ro/trn_rl_repo"):
    if os.path.isdir(_p) and _p not in sys.path:
        sys.path.insert(0, _p)

import ml_dtypes

import concourse.bacc as bacc
import concourse.bass as bass  # noqa: F401
import concourse.mybir as mybir
import concourse.tile as tile
from concourse.bass_utils import run_bass_kernel_spmd

F32 = mybir.dt.float32
F16 = mybir.dt.float16
BF16 = mybir.dt.bfloat16
F8 = mybir.dt.float8e3
F8E4 = mybir.dt.float8e4
NPF8 = ml_dtypes.float8_e3m4
NPF8E4 = ml_dtypes.float8_e4m3
NPBF16 = ml_dtypes.bfloat16

N, F_IN, HID, C, K = 8192, 512, 64, 16, 10
BASE_ALPHA = 0.5
JA, JB, JL, JR = 1.0, 1.0, -1.0, 1.0
NCORES = 8
SH = N // NCORES     # 1024
RCH = N // 128       # 64
MYR = SH // 128      # 8

SU, SZ, SY, SX, SW = 64.0, 8.0, 4.0, 2.0, 64.0

_CACHE = {}


def _jacobi_coef_rows(temp):
    """Host-precomputed per-channel coefficient rows, [30*C] packed.

    Rows 0 and 1 carry the extra Sy factor: the recurrence is linear in the
    xs state, so scaling the two seed rows scales hacc by Sy exactly.
    """
    a, b, l, r = JA, JB, JL, JR
    alphas = (BASE_ALPHA * np.tanh(np.asarray(temp, np.float64)))  # [C, K+1]
    rows = [SY * alphas[:, 0]]
    coef1 = (a - b) / 2 - (a + b + 2) / 2 * (l + r) / (r - l)
    coef2 = (a + b + 2) / (r - l)
    rows.append(SY * coef1 * alphas[:, 1])   # c1_0 (additive seed)
    rows.append(coef2 * alphas[:, 1])        # c1_1 (multiplies scaled xs)
    for L in range(2, K + 1):
        coef_l = 2 * L * (L + a + b) * (2 * L - 2 + a + b)
        c_lm1_1 = (2 * L + a + b - 1) * (2 * L + a + b) * (2 * L + a + b - 2)
        c_lm1_2 = (2 * L + a + b - 1) * (a ** 2 - b ** 2)
        c_lm2 = 2 * (L - 1 + a) * (L - 1 + b) * (2 * L + a + b)
        tmp1 = alphas[:, L - 1] * (c_lm1_1 / coef_l)
        tmp2 = alphas[:, L - 1] * (c_lm1_2 / coef_l)
        tmp3 = alphas[:, L - 1] * alphas[:, L - 2] * (c_lm2 / coef_l)
        rows.append(tmp1 * (2 / (r - l)))                    # t1
        rows.append(tmp1 * ((r + l) / (r - l)) + tmp2)       # t2
        rows.append(tmp3)                                    # t3
    packed = np.concatenate(rows).astype(np.float32).reshape(1, 30 * C)
    return np.ascontiguousarray(np.repeat(packed, 128, axis=0))


def _bc(ap, shape, axis=1):
    while ap.ndim < len(shape):
        ap = ap.unsqueeze(axis)
    return ap.broadcast_to(shape)


def _build():
    nc = bacc.Bacc("TRN2", target_bir_lowering=False, debug=False)

    u1d = nc.dram_tensor("u1d", [N, SH], F8E4, kind="ExternalInput")
    u2d = nc.dram_tensor("u2d", [N, SH], F8E4, kind="ExternalInput")
    xd = nc.dram_tensor("xd", [4096, 1024], F8E4, kind="ExternalInput")
    e_sh = nc.dram_tensor("e_shard", [MYR, 128], F32, kind="ExternalInput")
    w1r = nc.dram_tensor("w1r", [128, 4 * HID], F8E4, kind="ExternalInput")
    w2d = nc.dram_tensor("w2d", [HID, C], BF16, kind="ExternalInput")
    b1c = nc.dram_tensor("b1c", [HID, 1], F32, kind="ExternalInput")
    b2c = nc.dram_tensor("b2c", [1, C], F16, kind="ExternalInput")
    csd = nc.dram_tensor("csd", [1, SH], F16, kind="ExternalInput")
    jcd = nc.dram_tensor("jcd", [128, 30 * C], F32, kind="ExternalInput")
    id16x4d = nc.dram_tensor("id16x4d", [128, C], F32, kind="ExternalInput")
    idgd = nc.dram_tensor("idgd", [C, C], F16, kind="ExternalInput")
    idod = nc.dram_tensor("idod", [C, C], F16, kind="ExternalInput")
    warmd = nc.dram_tensor("warmd", [1, C], F32, kind="ExternalInput")
    out_sh = nc.dram_tensor("out_shard", [SH, C], F32, kind="ExternalOutput")

    rg = [list(range(NCORES))]

    with nc.allow_low_precision(reason="fp8 matmul path"), \
         tile.TileContext(nc) as tc:
        with (
            tc.tile_pool(name="dram", bufs=1, space="DRAM") as dram,
            tc.tile_pool(name="consts", bufs=1) as cp,
            tc.tile_pool(name="persist", bufs=1) as pp,
            tc.tile_pool(name="u1p", bufs=8) as u1p,
            tc.tile_pool(name="xqp", bufs=8) as xp,
            tc.tile_pool(name="small", bufs=4) as sp,
        ):
            warm_in = dram.tile([1, C], F32)
            warm_out = dram.tile([NCORES, C], F32, addr_space="Shared")
            ag_in1 = dram.tile([128, MYR * C // 2], F8E4)
            ag_in2 = dram.tile([128, MYR * C // 2], F8E4)
            ag_out1 = dram.tile([NCORES * 128, MYR * C // 2], F8E4,
                                addr_space="Shared")
            ag_out2 = dram.tile([NCORES * 128, MYR * C // 2], F8E4,
                                addr_space="Shared")

            id16x4 = cp.tile_from(id16x4d[:])
            idg = cp.tile_from(idgd[:])
            ido = cp.tile_from(idod[:])
            jc = cp.tile_from(jcd[:])
            w1 = cp.tile_from(w1r[:])
            w2 = cp.tile_from(w2d[:])
            b1 = cp.tile_from(b1c[:])
            b2 = cp.tile_from(b2c[:])
            cs = cp.tile_from(csd[:])
            e_row = cp.tile_from(e_sh[:])

            zsb = pp.tile([128, RCH, C], F8E4)
            Ysb_a = pp.tile([128, NCORES, MYR // 2, C], F8E4)
            Ysb_b = pp.tile([128, NCORES, MYR // 2, C], F8E4)
            ysb = pp.tile([128, MYR, C], F8E4)
            e_col = pp.tile([128, MYR], F32)
            hacc = pp.tile([128, MYR, C], F32)
            xs_a = pp.tile([128, MYR, C], F32)
            xs_b = pp.tile([128, MYR, C], F32)
            htmp = pp.tile([128, MYR, C], F32)
            htmp2 = pp.tile([128, MYR, C], F32)
            gsb = pp.tile([16, SH], F16)
            osb = pp.tile([16, SH], F16)
            u2t = pp.tile([128, RCH, SH], F8E4)
            smf = pp.tile([128, MYR, C], F32)
            smf2 = pp.tile([128, MYR, C], F32)
            et2 = pp.tile([128, MYR, C], F32)
            smout = pp.tile([128, MYR, C], F32)
            mnegs = pp.tile([128, MYR], F32)
            ssum = pp.tile([128, MYR], F32)
            lns = pp.tile([128, MYR], F32)

            # warm the CC stream: absorbs the post-barrier first-op delay
            nc.gpsimd.dma_start(out=warm_in[:], in_=warmd[:])
            nc.gpsimd.collective_compute(
                "AllGather", mybir.AluOpType.bypass, replica_groups=rg,
                ins=[warm_in.opt()], outs=[warm_out.opt()],
            )

            # x first on the fast queue (z gates GEMM1 tail), then u1, u2
            xq_t = []
            for ch in range(8):
                xTq = xp.tile([128, 4, 1024], F8E4, tag="xq")
                nc.sync.dma_start(
                    out=xTq[:],
                    in_=xd[512 * ch:512 * (ch + 1), :]
                    .rearrange("(p a) r -> p a r", p=128))
                xq_t.append(xTq)
            u1_t = []
            for t in range(8):
                ut = u1p.tile([128, 8, SH], F8E4, tag="u1")
                nc.sync.dma_start(
                    out=ut[:],
                    in_=u1d[1024 * t:1024 * (t + 1), :]
                    .rearrange("(p j) s -> p j s", p=128),
                )
                u1_t.append(ut)
            for t in range(8):
                nc.sync.dma_start(
                    out=u2t[:, 8 * t:8 * (t + 1), :],
                    in_=u2d[1024 * t:1024 * (t + 1), :]
                    .rearrange("(p j) s -> p j s", p=128),
                )

            # replicated MLP: full z (scaled by SZ), fp8, in [128, RCH, C]
            with tc.tile_pool(name="ppre", bufs=1, space="PSUM") as ppre, \
                 tc.tile_pool(name="mlp", bufs=2) as mp:
                pet = ppre.tile([128, MYR], F32, tag="ptmp", bufs=2)
                nc.tensor.transpose(pet[:], e_row[:], id16x4[0:MYR, 0:MYR])
                nc.scalar.copy(e_col[:], pet[:])
                for ch in range(8):
                    xTq = xq_t[ch]
                    ph = ppre.tile([HID, 1024], F32, tag="ph", bufs=1)
                    w1v = w1[:].rearrange("p (f h) -> p f h", h=HID)
                    for half in range(2):
                        for fb in range(0, 4, 2):
                            nc.tensor.matmul(
                                ph[:, 512 * half:512 * (half + 1)],
                                lhsT=w1v[:, fb:fb + 2, :],
                                rhs=xTq[:, fb:fb + 2, 512 * half:512 * (half + 1)],
                                perf_mode=mybir.MatmulPerfMode.DoubleRow,
                                start=(fb == 0), stop=(fb == 2),
                            )
                    hq = mp.tile([HID, 1024], BF16, tag="hq")
                    nc.scalar.activation(hq[:], ph[:],
                                         mybir.ActivationFunctionType.Relu,
                                         bias=b1[:, 0:1], scale=1.0 / (SX * SW))
                    pzt8 = ppre.tile([128, 8, C], F32, tag="pzt8", bufs=2)
                    for j in range(8):
                        nc.tensor.matmul(pzt8[:, j, :],
                                         lhsT=hq[:, 128 * j:128 * (j + 1)],
                                         rhs=w2[:], start=True, stop=True)
                    nc.scalar.copy(zsb[:, 8 * ch:8 * (ch + 1), :], pzt8[:])
            ev = _bc(e_col[:], (128, MYR, C), axis=2)

            def jrow(i):
                return _bc(jc[:, i * C:(i + 1) * C], (128, MYR, C))

            nc.vector.tensor_copy(xs_a[:], jrow(0))
            nc.vector.tensor_mul(htmp[:], xs_a[:], ev)
            nc.vector.tensor_mul(htmp[:], htmp[:], jrow(2))
            nc.vector.tensor_add(xs_b[:], htmp[:], jrow(1))
            nc.vector.tensor_add(hacc[:], xs_a[:], xs_b[:])
            xm2, xm1 = xs_a, xs_b
            for L in range(2, K + 1):
                r0 = 3 + 3 * (L - 2)
                nc.vector.tensor_mul(htmp[:], xm1[:], ev)
                nc.vector.tensor_mul(htmp[:], htmp[:], jrow(r0))
                nc.vector.tensor_mul(htmp2[:], xm1[:], jrow(r0 + 1))
                nc.vector.tensor_sub(htmp[:], htmp[:], htmp2[:])
                nc.vector.tensor_mul(htmp2[:], xm2[:], jrow(r0 + 2))
                nc.vector.tensor_sub(xm2[:], htmp[:], htmp2[:])
                nc.vector.tensor_add(hacc[:], hacc[:], xm2[:])
                xm2, xm1 = xm1, xm2

            with (
                tc.tile_pool(name="pmain", bufs=1, space="PSUM") as pm,
            ):
                # GEMM1: G^T * (SU*SZ) in two psum banks [16, 512]
                pg0 = pm.tile([C, 512], F32, tag="pg0")
                pg1 = pm.tile([C, 512], F32, tag="pg1")
                DR = mybir.MatmulPerfMode.DoubleRow
                for t in range(8):
                    ut = u1_t[t]
                    for j in range(0, 8, 2):
                        g = 8 * t + j
                        nc.tensor.matmul(
                            pg0[:], lhsT=zsb[:, g:g + 2, :],
                            rhs=ut[:, j:j + 2, 0:512], perf_mode=DR,
                            start=(g == 0), stop=(g == RCH - 2),
                            skip_group_check=True)
                        nc.tensor.matmul(
                            pg1[:], lhsT=zsb[:, g:g + 2, :],
                            rhs=ut[:, j:j + 2, 512:1024], perf_mode=DR,
                            start=(g == 0), stop=(g == RCH - 2),
                            skip_group_check=True)
                nc.scalar.copy(gsb[:, 0:512], pg0[:])
                nc.scalar.copy(gsb[:, 512:1024], pg1[:])

                # descale-transpose G blocks, multiply by hacc (*SY) -> y fp8
                for j in range(MYR):
                    pGy = pm.tile([128, C], F32, tag="pt", bufs=2)
                    nc.tensor.matmul(
                        pGy[:], lhsT=cs[0:1, 128 * j:128 * (j + 1)], rhs=b2[:],
                        start=True, stop=False, skip_group_check=True)
                    nc.tensor.matmul(
                        pGy[:], lhsT=gsb[:, 128 * j:128 * (j + 1)], rhs=idg[:],
                        start=False, stop=True, skip_group_check=True)
                    nc.vector.tensor_mul(ysb[:, j, :], pGy[:], hacc[:, j, :])
                    if j == MYR // 2 - 1:
                        nc.gpsimd.dma_start(
                            out=ag_in1[:].rearrange("p (j c) -> p j c", c=C),
                            in_=ysb[:, 0:MYR // 2, :])
                        nc.gpsimd.collective_compute(
                            "AllGather", mybir.AluOpType.bypass,
                            replica_groups=rg,
                            ins=[ag_in1.opt()], outs=[ag_out1.opt()])
                    elif j == MYR - 1:
                        nc.gpsimd.dma_start(
                            out=ag_in2[:].rearrange("p (j c) -> p j c", c=C),
                            in_=ysb[:, MYR // 2:MYR, :])
                        nc.gpsimd.collective_compute(
                            "AllGather", mybir.AluOpType.bypass,
                            replica_groups=rg,
                            ins=[ag_in2.opt()], outs=[ag_out2.opt()])
                nc.sync.dma_start(
                    out=Ysb_a[:].rearrange("p r j c -> p r (j c)"),
                    in_=ag_out1[:].rearrange("(r p) w -> p r w", p=128))
                nc.sync.dma_start(
                    out=Ysb_b[:].rearrange("p r j c -> p r (j c)"),
                    in_=ag_out2[:].rearrange("(r p) w -> p r w", p=128))

                # GEMM2: out^T * (SU*SY); bank-major so half the tail
                # overlaps the second accumulation pass
                po0 = pm.tile([C, 512], F32, tag="po0")
                po1 = pm.tile([C, 512], F32, tag="po1")
                for h, po in ((0, po0), (1, po1)):
                    for half, Yt in ((0, Ysb_a), (1, Ysb_b)):
                        for r in range(NCORES):
                            for jp in range(0, MYR // 2, 2):
                                g = 8 * r + 4 * half + jp
                                nc.tensor.matmul(
                                    po[:], lhsT=Yt[:, r, jp:jp + 2, :],
                                    rhs=u2t[:, g:g + 2, 512 * h:512 * (h + 1)],
                                    perf_mode=DR,
                                    start=(half == 0 and r == 0 and jp == 0),
                                    stop=(half == 1 and r == NCORES - 1
                                          and jp == MYR // 2 - 2),
                                    skip_group_check=True)
                    if h == 0:
                        nc.scalar.copy(osb[:, 0:512], po0[:])
                    else:
                        nc.vector.tensor_copy(osb[:, 512:1024], po1[:])
                    for jj in range(4):
                        j = 4 * h + jj
                        pf = pm.tile([128, C], F32, tag="pt", bufs=2)
                        nc.tensor.matmul(
                            pf[:], lhsT=osb[:, 128 * j:128 * (j + 1)], rhs=ido[:],
                            start=True, stop=True)
                        if j % 2 == 0:
                            nc.scalar.copy(smf[:, j, :], pf[:])
                        else:
                            nc.vector.tensor_copy(smf[:, j, :], pf[:])
                    # batched log_softmax for this half of the rows
                    sl = slice(4 * h, 4 * h + 4)
                    nc.vector.tensor_reduce(
                        out=mnegs[:, sl], in_=smf[:, sl, :],
                        op=mybir.AluOpType.max, axis=mybir.AxisListType.X,
                        negate=True)
                    nc.vector.tensor_add(
                        smf2[:, sl, :], smf[:, sl, :],
                        _bc(mnegs[:, sl], (128, 4, C), axis=2))
                    nc.scalar.activation(et2[:, sl, :], smf2[:, sl, :],
                                         mybir.ActivationFunctionType.Exp)
                    nc.vector.tensor_reduce(
                        out=ssum[:, sl], in_=et2[:, sl, :],
                        op=mybir.AluOpType.add, axis=mybir.AxisListType.X)
                    nc.scalar.activation(lns[:, sl], ssum[:, sl],
                                         mybir.ActivationFunctionType.Ln)
                    nc.vector.tensor_sub(
                        smout[:, sl, :], smf2[:, sl, :],
                        _bc(lns[:, sl], (128, 4, C), axis=2))
                    nc.scalar.dma_start(
                        out=out_sh[:].rearrange("(j p) c -> p j c", p=128)[:, sl, :],
                        in_=smout[:, sl, :])

    nc.compile()
    return nc


def _pack_rows(a):
    """Reorder [8192, 1024] rows so tile partition lines are 8KB contiguous:
    packed[1024*t + 8*p + j] = a[1024*t + 128*j + p]."""
    return np.ascontiguousarray(
        a.reshape(8, 8, 128, 1024).transpose(0, 2, 1, 3).reshape(8192, 1024))


def _prep_inputs(origin_e, U, x, W1, b1, W2, b2, temp):
    origin_e = np.ascontiguousarray(np.asarray(origin_e, np.float32))
    U = np.asarray(U, np.float32)
    x = np.asarray(x, np.float32)
    W1 = np.asarray(W1, np.float32)
    b1 = np.asarray(b1, np.float32)
    W2 = np.asarray(W2, np.float32)
    b2 = np.asarray(b2, np.float32)

    jc = _jacobi_coef_rows(temp)
    id16 = np.zeros((128, C), np.float32)
    for k in range(4):
        id16[32 * k:32 * k + C, :] = np.eye(C, dtype=np.float32)
    idg = (np.eye(C, dtype=np.float32) / (SU * SZ)).astype(np.float16)
    ido = (np.eye(C, dtype=np.float32) / (SU * SY)).astype(np.float16)
    w1r = np.ascontiguousarray(
        np.clip(W1 * SW, -240, 240).reshape(4, 128, HID).transpose(1, 0, 2)
        .reshape(128, 4 * HID)).astype(NPF8E4)
    shared = {
        "w1r": w1r, "w2d": np.ascontiguousarray(W2 * SZ).astype(NPBF16),
        "b1c": np.ascontiguousarray(b1.reshape(HID, 1)),
        "b2c": np.ascontiguousarray(b2.reshape(1, C)).astype(np.float16),
        "jcd": jc, "id16x4d": id16, "idgd": idg, "idod": ido,
        "warmd": np.zeros((1, C), np.float32),
        "xd": np.ascontiguousarray(
            np.clip(x.T * SX, -240, 240).astype(NPF8E4).reshape(4, 128, 8, 1024)
            .transpose(2, 1, 0, 3).reshape(4096, 1024)),
    }
    Us = np.clip(U * SU, -240, 240).astype(NPF8E4)
    in_maps = []
    for i in range(NCORES):
        m = dict(shared)
        m["u1d"] = _pack_rows(Us[:, i * SH:(i + 1) * SH])
        m["u2d"] = _pack_rows(np.ascontiguousarray(Us[i * SH:(i + 1) * SH, :].T))
        m["csd"] = np.ascontiguousarray(
            U[:, i * SH:(i + 1) * SH].sum(axis=0).reshape(1, SH)
        ).astype(np.float16)
        m["e_shard"] = np.ascontiguousarray(
            origin_e[i * SH:(i + 1) * SH].reshape(MYR, 128))
        in_maps.append(m)
    return in_maps


def _get_program():
    if "nc" not in _CACHE:
        _CACHE["nc"] = _build()
    return _CACHE["nc"]


def run(inputs, trace=False, **kw):
    nc = _get_program()
    in_maps = _prep_inputs(**inputs)
    res = run_bass_kernel_spmd(nc, in_maps, core_ids=list(range(NCORES)),
                               trace=trace, **kw)
    out = np.concatenate([res.results[i]["out_shard"] for i in range(NCORES)], axis=0)
    return out, res


def kernel(origin_e, U, x, W1, b1, W2, b2, temp):
    out, _ = run(dict(origin_e=origin_e, U=U, x=x, W1=W1, b1=b1, W2=W2,
                      b2=b2, temp=temp))
    return out
